# revision 1
# baseline (speedup 1.0000x reference)
"""GCN message-passing kernel for Trainium2 (8 NeuronCores, Bass/Tile).

Strategy (dest-sharded, gather-based):
  - 200k nodes split into 8 shards of 25k destination rows (one per core).
  - out = node_norm(gcn + fnn) where gcn = (S @ x) @ W_gcn + b_gcn: the GCN
    aggregation commutes with the weight matmul, so each core gathers raw x
    rows for its edges (dma_gather, int16 indices over 8 source windows of
    25k rows), scales by the per-edge norm, and scatter-adds (dma_scatter_add)
    into a per-core HBM accumulator. Duplicate destinations within a scatter
    call race the RMW, so edges are reordered into "rounds" with unique
    destinations per call (host-side lexsort); calls are WAW-serialized.
  - Self loops fold in as agg2 = agg + dinv2*x (host supplies (dinv2*x)^T).
  - Phase C runs feature-major so biases are per-partition: g^T = Wg^T@agg2^T,
    f^T = Lrelu(Wf^T@x^T + bf), s^T = (g^T+bg)+f^T; PE transposes s back and
    ones-matmuls give row sums of s and s^2 for the node-norm.
Inputs are replicated per core (full x for gathers + per-core x^T slices).
"""

import numpy as np

N_SRC = 100000
N_TAR = 100000
N = N_SRC + N_TAR
D = 128
NCORE = 8
SH = N // NCORE          # 25000 dest rows per core
NWIN = 8                 # source windows of SH rows (int16 index limit)
HALF = 12544             # dest rows per half (2*HALF = 25088 >= SH)
SPAD = 2 * HALF          # padded per-core row count (196 tiles of 128)
NT_HALF = HALF // 128    # 98 tiles per half
TW = 2                   # row-tiles per phase-C iteration
EPS = 1e-6
NEG = 0.01

_CACHE = {}
PHASE_B = True          # debug bisect flags
PHASE_C = True


def _wrap16(a):
    w = a.reshape(-1, 16).T
    return np.tile(w, (8, 1)).astype(np.int16, copy=True)


def _wrap128(a):
    return np.ascontiguousarray(a.reshape(-1, 128).T.astype(np.float32))


def _host_prep(x, edge_index, edge_weight):
    """Bucket, dest-sort, round-split and pad all edges. Returns per-core
    per-(half,window) wrapped index/dest/norm arrays + shared round layout."""
    row = np.asarray(edge_index[0], dtype=np.int64)
    col = np.asarray(edge_index[1], dtype=np.int64)
    w = np.asarray(edge_weight, dtype=np.float32)

    deg = np.bincount(col, weights=w.astype(np.float64), minlength=N)
    deg = (deg + 1.0).astype(np.float32)
    dinv = (1.0 / np.sqrt(deg)).astype(np.float32)
    norm = dinv[row] * w * dinv[col]

    core = col // SH
    dstl = col - core * SH
    half = dstl // HALF
    dsth = (dstl - half * HALF).astype(np.int64)
    win = row // SH
    idxl = (row - win * SH).astype(np.int16)

    bucket = (core * 2 + half) * NWIN + win          # 0..127
    # group by (bucket, dest); rank within each (bucket,dest) run = round
    order1 = np.lexsort((dsth, bucket))
    bs, ds = bucket[order1], dsth[order1]
    comb = bs * 16384 + ds
    change = np.empty(len(comb), dtype=bool)
    change[0] = True
    change[1:] = comb[1:] != comb[:-1]
    runstart = np.maximum.accumulate(np.where(change, np.arange(len(comb)), 0))
    rnd = np.arange(len(comb)) - runstart
    order2 = np.lexsort((ds, rnd, bs))               # (bucket, round, dest)
    perm = order1[order2]
    bs2, rnd2 = bucket[perm], rnd[order2]

    nr = int(rnd.max()) + 1
    # per (bucket, round) counts -> padded sizes shared across cores
    cnt = np.bincount(bs2 * nr + rnd2, minlength=128 * nr).reshape(128, nr)
    cnt = cnt.reshape(NCORE, 2 * NWIN, nr)
    pad = ((cnt.max(axis=0) + 127) // 128) * 128      # [2*NWIN, nr]
    rounds = []                                       # per (h,s): [(start,cnt)]
    for hs in range(2 * NWIN):
        r_list, start = [], 0
        for r in range(nr):
            c = int(pad[hs, r])
            if c == 0:
                continue
            r_list.append((start, c))
            start += c
        rounds.append((r_list, start))                # start == B[hs]

    # slice per-core data and pad
    idx_s, dst_s, nrm_s = idxl[perm], dsth[perm], norm[perm]
    off = np.zeros(128 * nr + 1, dtype=np.int64)
    np.cumsum(np.bincount(bs2 * nr + rnd2, minlength=128 * nr), out=off[1:])

    per_core = []
    for k in range(NCORE):
        bufs = {}
        for hs in range(2 * NWIN):
            r_list, B = rounds[hs]
            ii = np.zeros(B, dtype=np.int16)
            dd = np.zeros(B, dtype=np.int64)
            nn_ = np.zeros(B, dtype=np.float32)
            # pads: index 0 (harmless read), dest -> dump rows, norm 0
            dd[:] = HALF + (np.arange(B) % 128)
            b_id = (k * 2 * NWIN) + hs
            pos = 0
            for r_i, (start, c) in enumerate(r_list):
                lo, hi = off[b_id * nr + r_i], off[b_id * nr + r_i + 1]
                n_real = hi - lo
                ii[start:start + n_real] = idx_s[lo:hi]
                dd[start:start + n_real] = dst_s[lo:hi]
                nn_[start:start + n_real] = nrm_s[lo:hi]
                pos += c
            bufs[hs] = (_wrap16(ii), _wrap16(dd.astype(np.int16)), _wrap128(nn_))
        per_core.append(bufs)
    return per_core, rounds, dinv


def _build_program(rounds):
    from concourse import bacc, mybir, tile

    f32 = mybir.dt.float32
    i16 = mybir.dt.int16
    nc = bacc.Bacc(None, num_swdge_queues=2)

    xfull = nc.dram_tensor("xfull", [N, D], f32, kind="ExternalInput")
    xT = nc.dram_tensor("xT", [D, SPAD], f32, kind="ExternalInput")
    xdT = nc.dram_tensor("xdT", [D, SPAD], f32, kind="ExternalInput")
    Wg_d = nc.dram_tensor("Wg", [D, D], f32, kind="ExternalInput")
    Wf_d = nc.dram_tensor("Wf", [D, D], f32, kind="ExternalInput")
    bg_d = nc.dram_tensor("bg", [D, 1], f32, kind="ExternalInput")
    bf_d = nc.dram_tensor("bf", [D, 1], f32, kind="ExternalInput")
    id_d = nc.dram_tensor("ident", [D, D], f32, kind="ExternalInput")
    on_d = nc.dram_tensor("ones", [D, 1], f32, kind="ExternalInput")
    ep_d = nc.dram_tensor("eps", [D, 1], f32, kind="ExternalInput")
    zf_d = nc.dram_tensor("zfill", [128, 2048], f32, kind="ExternalInput")
    meta = {}
    for hs in range(2 * NWIN):
        _, B = rounds[hs]
        meta[hs] = (
            nc.dram_tensor(f"idx{hs}", [128, B // 16], i16, kind="ExternalInput"),
            nc.dram_tensor(f"dst{hs}", [128, B // 16], i16, kind="ExternalInput"),
            nc.dram_tensor(f"nrm{hs}", [128, B // 128], f32, kind="ExternalInput"),
        )
    agg = [nc.dram_tensor(f"agg{h}", [HALF + 128, D], f32) for h in range(2)]
    out_d = nc.dram_tensor("out", [SH, D], f32, kind="ExternalOutput")
    chin_d = nc.dram_tensor("chin", [128, 128], f32, kind="ExternalInput")
    chout_d = nc.dram_tensor("chout", [128, 128], f32, kind="ExternalOutput")

    AOp = mybir.AluOpType
    AF = mybir.ActivationFunctionType

    with tile.TileContext(nc) as tc:
        with tc.tile_pool(name="const", bufs=1) as cpool, \
             tc.tile_pool(name="metap", bufs=4) as mpool, \
             tc.tile_pool(name="gath", bufs=4) as gpool, \
             tc.tile_pool(name="work", bufs=4) as wpool, \
             tc.tile_pool(name="psum", bufs=2, space="PSUM") as ppool, \
             tc.tile_pool(name="psum1", bufs=1, space="PSUM") as ppool1:

            Wg_t = cpool.tile([D, D], f32, tag="wg")
            Wf_t = cpool.tile([D, D], f32, tag="wf")
            bg_t = cpool.tile([D, 1], f32, tag="bg")
            bf_t = cpool.tile([D, 1], f32, tag="bf")
            id_t = cpool.tile([D, D], f32, tag="id")
            on_t = cpool.tile([D, 1], f32, tag="on")
            ep_t = cpool.tile([D, 1], f32, tag="ep")
            zf_t = cpool.tile([128, 2048], f32, tag="zf")
            for t, d in [(Wg_t, Wg_d), (Wf_t, Wf_d), (bg_t, bg_d), (bf_t, bf_d),
                         (id_t, id_d), (on_t, on_d), (ep_t, ep_d), (zf_t, zf_d)]:
                nc.sync.dma_start(out=t[:], in_=d[:])
            ch_t = cpool.tile([128, 128], f32, tag="ch")
            nc.sync.dma_start(out=ch_t[:], in_=chin_d[:])
            nc.sync.dma_start(out=chout_d[:], in_=ch_t[:])

            # zero-fill both agg halves (content of zf tile is zeros)
            for h in range(2):
                r0 = 0
                while r0 < HALF + 128:
                    nr_ = min(2048, HALF + 128 - r0)
                    nc.sync.dma_start(out=agg[h][r0:r0 + nr_, :],
                                      in_=zf_t[:, :nr_])
                    r0 += nr_

            # ---- Phase B: gather / scale / scatter per (half, window) ----
            for h in range(2 if PHASE_B else 0):
                for s in range(NWIN):
                    hs = h * NWIN + s
                    r_list, B = rounds[hs]
                    B128 = B // 128
                    idx_t = mpool.tile([128, B // 16], i16, tag="idx")
                    dst_t = mpool.tile([128, B // 16], i16, tag="dst")
                    nrm_t = mpool.tile([128, B128], f32, tag="nrm")
                    nc.sync.dma_start(out=idx_t[:], in_=meta[hs][0][:])
                    nc.sync.dma_start(out=dst_t[:], in_=meta[hs][1][:])
                    nc.sync.dma_start(out=nrm_t[:], in_=meta[hs][2][:])
                    g_t = gpool.tile([128, B128, D], f32, tag="g")
                    nc.gpsimd.dma_gather(
                        out_ap=g_t[:], in_ap=xfull[s * SH:(s + 1) * SH, :],
                        idxs_ap=idx_t[:], num_idxs=B, num_idxs_reg=B,
                        elem_size=D, queue_num=0, single_packet=False)
                    nc.vector.tensor_mul(
                        g_t[:], g_t[:], nrm_t[:].to_broadcast((128, B128, D)))
                    for (start, c) in r_list:
                        nc.gpsimd.dma_scatter_add(
                            out_ap=agg[h][:], idxs_ap=dst_t[:, start // 16:(start + c) // 16],
                            in_ap=g_t[:, start // 128:(start + c) // 128, :],
                            num_idxs=c, num_idxs_reg=c, elem_size=D,
                            queue_num=1, single_packet=False)

            # ---- Phase C: per TW-tile group ----
            for h in range(2 if PHASE_C else 0):
                for ti in range(0, NT_HALF, TW):
                    tw = min(TW, NT_HALF - ti)
                    W = tw * 128
                    c0 = h * HALF + ti * 128          # global padded row/col
                    base = ti * 128
                    A_t = wpool.tile([128, tw, D], f32, tag="A")
                    nc.sync.dma_start(
                        out=A_t[:],
                        in_=agg[h][base:base + W, :].rearrange(
                            "(j p) d -> p j d", p=128))
                    xT_t = wpool.tile([D, W], f32, tag="xT")
                    xdT_t = wpool.tile([D, W], f32, tag="xdT")
                    nc.sync.dma_start(out=xT_t[:], in_=xT[:, c0:c0 + W])
                    nc.sync.dma_start(out=xdT_t[:], in_=xdT[:, c0:c0 + W])

                    ATp = ppool.tile([D, W], f32, tag="ATp")
                    for j in range(tw):
                        nc.tensor.transpose(ATp[:, j * 128:(j + 1) * 128],
                                            A_t[:, j, :], id_t[:])
                    ATs = wpool.tile([D, W], f32, tag="ATs")
                    nc.scalar.copy(out=ATs[:], in_=ATp[:])
                    A2T = wpool.tile([D, W], f32, tag="A2T")
                    nc.vector.tensor_add(A2T[:], ATs[:], xdT_t[:])

                    GpT = ppool.tile([D, W], f32, tag="GpT")
                    nc.tensor.matmul(GpT[:], Wg_t[:], A2T[:], start=True, stop=True)
                    FpT = ppool.tile([D, W], f32, tag="FpT")
                    nc.tensor.matmul(FpT[:], Wf_t[:], xT_t[:], start=True, stop=True)
                    fT = wpool.tile([D, W], f32, tag="fT")
                    nc.scalar.activation(fT[:], FpT[:], AF.Prelu,
                                         bias=bf_t[:], scale=1.0, alpha=NEG)
                    sT = wpool.tile([D, W], f32, tag="sT")
                    nc.vector.scalar_tensor_tensor(sT[:], GpT[:], bg_t[:], fT[:],
                                                   op0=AOp.add, op1=AOp.add)
                    sqT = wpool.tile([D, W], f32, tag="sqT")
                    nc.scalar.activation(sqT[:], sT[:], AF.Square)

                    # transpose back + per-tile row sums of s and s^2
                    s_bk = ppool1.tile([D, W], f32, tag="s_bk")
                    sums = ppool1.tile([D, 2 * TW], f32, tag="sums")
                    for j in range(tw):
                        sl = sT[:, j * 128:(j + 1) * 128]
                        ql = sqT[:, j * 128:(j + 1) * 128]
                        nc.tensor.transpose(s_bk[:, j * 128:(j + 1) * 128], sl, id_t[:])
                        nc.tensor.matmul(sums[:, j:j + 1], sl, on_t[:],
                                         start=True, stop=True)
                        nc.tensor.matmul(sums[:, TW + j:TW + j + 1], ql, on_t[:],
                                         start=True, stop=True)

                    means = wpool.tile([D, TW], f32, tag="means")
                    nc.vector.tensor_scalar_mul(means[:, :tw], sums[:, :tw], 1.0 / D)
                    msq = wpool.tile([D, TW], f32, tag="msq")
                    nc.vector.tensor_mul(msq[:, :tw], means[:, :tw], means[:, :tw])
                    veps = wpool.tile([D, TW], f32, tag="veps")
                    nc.vector.scalar_tensor_tensor(
                        veps[:, :tw], sums[:, TW:TW + tw], 1.0 / D, msq[:, :tw],
                        op0=AOp.mult, op1=AOp.subtract)
                    std = wpool.tile([D, TW], f32, tag="std")
                    nc.scalar.activation(std[:, :tw], veps[:, :tw], AF.Sqrt,
                                         bias=ep_t[:])
                    rstd = wpool.tile([D, TW], f32, tag="rstd")
                    nc.vector.reciprocal(rstd[:, :tw], std[:, :tw])

                    o1 = wpool.tile([128, tw, D], f32, tag="o1")
                    for j in range(tw):
                        nc.vector.tensor_scalar(o1[:, j, :],
                                                s_bk[:, j * 128:(j + 1) * 128],
                                                means[:, j:j + 1], rstd[:, j:j + 1],
                                                op0=AOp.subtract, op1=AOp.mult)
                    o2 = wpool.tile([128, tw, D], f32, tag="o2")
                    nc.scalar.activation(o2[:], o1[:], AF.Prelu, alpha=NEG)
                    rg = c0                            # first padded row of group
                    n_out = min(W, max(0, SH - rg))
                    if n_out == W:
                        nc.sync.dma_start(
                            out=out_d[rg:rg + W, :].rearrange(
                                "(j p) d -> p j d", p=128),
                            in_=o2[:])
                    else:
                        for j in range(tw):
                            rj = rg + j * 128
                            nj = min(128, max(0, SH - rj))
                            if nj > 0:
                                nc.sync.dma_start(out=out_d[rj:rj + nj, :],
                                                  in_=o2[:nj, j, :])
            if not PHASE_C:
                dummy = cpool.tile([128, 128], f32, tag="dummy")
                nc.vector.memset(dummy[:], 0.0)
                nc.sync.dma_start(out=out_d[0:128, :], in_=dummy[:])
    nc.finalize()
    return nc


def _plan(x_src, x_tar, edge_index, edge_weight, W_gcn, b_gcn, W_fnn, b_fnn):
    """Host prep + (cached) program build. Returns (nc, in_maps, assemble)."""
    x = np.concatenate([np.asarray(x_src, np.float32),
                        np.asarray(x_tar, np.float32)], axis=0)
    per_core, rounds, dinv = _host_prep(x, edge_index, edge_weight)

    key = (PHASE_B, PHASE_C) + tuple(B for (_r, B) in rounds) + tuple(
        tuple(r) for (r, _B) in rounds)
    if key not in _CACHE:
        _CACHE[key] = _build_program(rounds)
    nc = _CACHE[key]

    ident = np.eye(D, dtype=np.float32)
    common = {
        "Wg": np.asarray(W_gcn, np.float32),
        "Wf": np.asarray(W_fnn, np.float32),
        "bg": np.asarray(b_gcn, np.float32).reshape(D, 1),
        "bf": np.asarray(b_fnn, np.float32).reshape(D, 1),
        "ident": ident,
        "ones": np.ones((D, 1), np.float32),
        "eps": np.full((D, 1), EPS, np.float32),
        "zfill": np.zeros((128, 2048), np.float32),
    }
    in_maps = []
    d2 = (dinv * dinv).astype(np.float32)
    for k in range(NCORE):
        xo = x[k * SH:(k + 1) * SH]
        xT_k = np.zeros((D, SPAD), np.float32)
        xT_k[:, :SH] = xo.T
        xdT_k = np.zeros((D, SPAD), np.float32)
        xdT_k[:, :SH] = (xo * d2[k * SH:(k + 1) * SH, None]).T
        m = dict(common)
        m["chin"] = np.zeros((128, 128), np.float32)
        m["xfull"] = x
        m["xT"] = xT_k
        m["xdT"] = xdT_k
        for hs in range(2 * NWIN):
            ii, dd, nn_ = per_core[k][hs]
            m[f"idx{hs}"] = ii
            m[f"dst{hs}"] = dd
            m[f"nrm{hs}"] = nn_
        in_maps.append(m)

    def assemble(results):
        full = np.concatenate([results[k]["out"] for k in range(NCORE)], axis=0)
        return full[:N_SRC, :], full[N_SRC:, :]

    return nc, in_maps, assemble


def kernel(x_src, x_tar, edge_index, edge_weight, W_gcn, b_gcn, W_fnn, b_fnn):
    from concourse.bass_utils import run_bass_kernel_spmd

    nc, in_maps, assemble = _plan(x_src, x_tar, edge_index, edge_weight,
                                  W_gcn, b_gcn, W_fnn, b_fnn)
    res = run_bass_kernel_spmd(nc, in_maps, list(range(NCORE)))
    return assemble(res.results)



# revision 8
# speedup vs baseline: 2.4393x; 2.4393x over previous
"""GCN message-passing kernel for Trainium2 (8 NeuronCores, Bass/Tile).

Strategy (dest-sharded, host-pregathered edge stream + PE segment-sum):
  - 200k nodes split into 8 shards of 25k destination rows (one per core);
    dest space padded to 25088 = 196 slots of 128 dests.
  - The GCN aggregation A[d] = sum_e norm_e x[src_e] is factored as
    norm_e = dinv[src] * w_e * dinv[dst]: dinv[src] is folded into a host
    tensor xs = dinv*x (bf16); w_e*dinv[dst] becomes the value of a per-tile
    one-hot matrix; the host pre-gathers xe[i] = xs[src_e] in (core, slot)-
    sorted order so the device streams it contiguously (no dma_gather).
    Each slot's messages are padded to whole 128-row tiles; tile counts are
    shared across cores (max over cores) so the SPMD program is uniform.
  - Per 128-msg tile, DVE/Pool builds onehot[m, d] = (iota==dst_m)*nrm_m
    (bf16); PE accumulates AggT[feat, dst] += xe_tile.T @ onehot into a
    per-quad (4 slots) PSUM bank.
  - Phase C per quad (feature-major): A2T = AggT + xdT (self loops,
    xdT = (dinv^2 x)^T), gT = Wg.T@A2T, fT = Wf.T@xT, f = Prelu(fT+bf),
    sT = gT+bg+f, s = sT.T per slot (PE transpose), row sums of s and s^2
    via ones-matmuls (s^2 on Pool), out = Prelu(rstd*s - rstd*mean).
"""

import numpy as np

N_SRC = 100000
N_TAR = 100000
N = N_SRC + N_TAR
D = 128
NCORE = 8
SH = N // NCORE          # 25000 dest rows per core
NSLOT = 196              # slots of 128 dests; 196*128 = 25088 >= 25000
SPAD = NSLOT * 128
NSEC = 7                 # sections of 28 slots (xe streaming granularity)
SSEC = NSLOT // NSEC     # 28 slots per section
QUAD = 4                 # slots per PSUM quad
NQUAD = SSEC // QUAD     # 7 quads per section
EPS = 1e-6
NEG = 0.01
OH_DVE_FRAC = 0.95       # fraction of one-hot builds on DVE (rest on Pool)

_CACHE = {}


def _bf16():
    import ml_dtypes
    return ml_dtypes.bfloat16


def _host_prep(x, edge_index, edge_weight):
    """Returns (ntile [NSLOT], per_core in_map fragments)."""
    bf16 = _bf16()
    row = np.asarray(edge_index[0], dtype=np.int64)
    col = np.asarray(edge_index[1], dtype=np.int64)
    w = np.asarray(edge_weight, dtype=np.float32)

    deg = np.bincount(col, weights=w.astype(np.float64), minlength=N)
    deg = (deg + 1.0).astype(np.float32)
    dinv = (1.0 / np.sqrt(deg)).astype(np.float32)

    xs16 = (x * dinv[:, None]).astype(bf16)          # dinv[src] folded in
    nrm2 = (w * dinv[col]).astype(np.float32)        # per-edge w_e*dinv[dst]

    core = col // SH
    dstl = col - core * SH
    slot = dstl >> 7                                 # 0..195
    dis = (dstl & 127).astype(np.float32)            # dest-in-slot

    key = core * NSLOT + slot
    cnt = np.bincount(key, minlength=NCORE * NSLOT).reshape(NCORE, NSLOT)
    cap = np.maximum(cnt.max(axis=0), 1).astype(np.int64)
    ntile = (cap + 127) // 128                       # tiles per slot (shared)
    capp = ntile * 128

    # slot start offsets (tile-aligned) within each section
    o_slot = np.zeros(NSLOT, dtype=np.int64)
    B_list = []
    for sec in range(NSEC):
        cq = capp[sec * SSEC:(sec + 1) * SSEC]
        off = np.concatenate([[0], np.cumsum(cq)])
        o_slot[sec * SSEC:(sec + 1) * SSEC] = off[:-1]
        B_list.append(int(off[-1]))

    # rank of each edge within its (core, slot) bucket
    order = np.argsort(key, kind="stable")
    ks = key[order]
    change = np.empty(len(ks), dtype=bool)
    change[0] = True
    change[1:] = ks[1:] != ks[:-1]
    runstart = np.maximum.accumulate(np.where(change, np.arange(len(ks)), 0))
    rank = np.arange(len(ks)) - runstart
    rank_e = np.empty(len(ks), dtype=np.int64)
    rank_e[order] = rank

    pos = o_slot[slot] + rank_e                      # row within section buf
    secs = slot // SSEC

    per_core = []
    for k in range(NCORE):
        mk = core == k
        m = {}
        for sec in range(NSEC):
            sel = mk & (secs == sec)
            B = B_list[sec]
            T = B // 128
            xe = np.zeros((B, D), dtype=bf16)
            xe[pos[sel]] = xs16[row[sel]]
            dstb = np.zeros(B, dtype=np.float32)
            dstb[pos[sel]] = dis[sel]
            nrmb = np.zeros(B, dtype=np.float32)
            nrmb[pos[sel]] = nrm2[sel]
            m[f"xe{sec}"] = np.ascontiguousarray(
                xe.reshape(T, 128, D).transpose(1, 0, 2).reshape(128, T * D))
            m[f"dst{sec}"] = np.ascontiguousarray(dstb.reshape(T, 128).T)
            m[f"nrm{sec}"] = np.ascontiguousarray(nrmb.reshape(T, 128).T)
        xk = x[k * SH:(k + 1) * SH]
        d2k = (dinv[k * SH:(k + 1) * SH] ** 2).astype(np.float32)
        xT = np.zeros((D, SPAD), dtype=bf16)
        xT[:, :SH] = xk.T.astype(bf16)
        xdT = np.zeros((D, SPAD), dtype=bf16)
        xdT[:, :SH] = (xk * d2k[:, None]).T.astype(bf16)
        m["xT"] = xT
        m["xdT"] = xdT
        per_core.append(m)

    return ntile, per_core


def _build_program(ntile):
    from concourse import bacc, mybir, tile

    f32 = mybir.dt.float32
    bf = mybir.dt.bfloat16
    nc = bacc.Bacc(None)

    T_list = [int(ntile[s * SSEC:(s + 1) * SSEC].sum()) for s in range(NSEC)]
    xe_d = [nc.dram_tensor(f"xe{s}", [128, T_list[s] * D], bf,
                           kind="ExternalInput") for s in range(NSEC)]
    dst_d = [nc.dram_tensor(f"dst{s}", [128, T_list[s]], f32,
                            kind="ExternalInput") for s in range(NSEC)]
    nrm_d = [nc.dram_tensor(f"nrm{s}", [128, T_list[s]], f32,
                            kind="ExternalInput") for s in range(NSEC)]
    xT_d = nc.dram_tensor("xT", [D, SPAD], bf, kind="ExternalInput")
    xdT_d = nc.dram_tensor("xdT", [D, SPAD], bf, kind="ExternalInput")
    Wg_d = nc.dram_tensor("Wg", [D, D], bf, kind="ExternalInput")
    Wf_d = nc.dram_tensor("Wf", [D, D], bf, kind="ExternalInput")
    bg_d = nc.dram_tensor("bg", [D, 1], f32, kind="ExternalInput")
    bf_d = nc.dram_tensor("bf", [D, 1], f32, kind="ExternalInput")
    io_d = nc.dram_tensor("iota", [128, 128], bf, kind="ExternalInput")
    id_d = nc.dram_tensor("ident", [D, D], f32, kind="ExternalInput")
    idb_d = nc.dram_tensor("identb", [D, D], bf, kind="ExternalInput")
    on_d = nc.dram_tensor("ones", [D, 1], f32, kind="ExternalInput")
    ep_d = nc.dram_tensor("eps", [D, 1], f32, kind="ExternalInput")
    out_d = nc.dram_tensor("out", [SH, D], f32, kind="ExternalOutput")
    chin_d = nc.dram_tensor("chin", [128, 128], f32, kind="ExternalInput")
    chout_d = nc.dram_tensor("chout", [128, 128], f32, kind="ExternalOutput")

    AOp = mybir.AluOpType
    AF = mybir.ActivationFunctionType
    W4 = QUAD * 128      # 512

    oh_count = [0]
    oh_total = int(ntile.sum())

    with tile.TileContext(nc) as tc:
        with tc.tile_pool(name="const", bufs=1) as cpool, \
             tc.tile_pool(name="xep", bufs=2) as xepool, \
             tc.tile_pool(name="metap", bufs=2) as mpool, \
             tc.tile_pool(name="ohp", bufs=4) as ohpool, \
             tc.tile_pool(name="xtp", bufs=2) as xtpool, \
             tc.tile_pool(name="work", bufs=2) as wpool, \
             tc.tile_pool(name="stagep", bufs=2) as stpool, \
             tc.tile_pool(name="statp", bufs=2) as statp, \
             tc.tile_pool(name="aggp", bufs=2, space="PSUM") as aggp, \
             tc.tile_pool(name="gp", bufs=1, space="PSUM") as gp, \
             tc.tile_pool(name="fp", bufs=1, space="PSUM") as fp, \
             tc.tile_pool(name="sp", bufs=2, space="PSUM") as sp, \
             tc.tile_pool(name="sumsp", bufs=1, space="PSUM") as sumsp:

            Wg_t = cpool.tile([D, D], bf, tag="wg")
            Wf_t = cpool.tile([D, D], bf, tag="wf")
            bg_t = cpool.tile([D, 1], f32, tag="bg")
            bf_t = cpool.tile([D, 1], f32, tag="bf")
            io_t = cpool.tile([128, 128], bf, tag="io")
            id_t = cpool.tile([D, D], f32, tag="id")
            idb_t = cpool.tile([D, D], bf, tag="idb")
            on_t = cpool.tile([D, 1], f32, tag="on")
            ep_t = cpool.tile([D, 1], f32, tag="ep")
            for t, d in [(Wg_t, Wg_d), (Wf_t, Wf_d), (bg_t, bg_d),
                         (bf_t, bf_d), (io_t, io_d), (id_t, id_d),
                         (idb_t, idb_d), (on_t, on_d), (ep_t, ep_d)]:
                nc.sync.dma_start(out=t[:], in_=d[:])
            ch_t = cpool.tile([128, 128], f32, tag="ch")
            nc.sync.dma_start(out=ch_t[:], in_=chin_d[:])
            nc.sync.dma_start(out=chout_d[:], in_=ch_t[:])

            for sec in range(NSEC):
                T = T_list[sec]
                xe_t = xepool.tile([128, T, D], bf, tag="xe")
                nc.sync.dma_start(out=xe_t[:], in_=xe_d[sec][:].rearrange(
                    "p (t d) -> p t d", d=D))
                dst_t = mpool.tile([128, T], f32, tag="dst")
                nrm_t = mpool.tile([128, T], f32, tag="nrm")
                nc.sync.dma_start(out=dst_t[:], in_=dst_d[sec][:])
                nc.sync.dma_start(out=nrm_t[:], in_=nrm_d[sec][:])

                jbase = 0           # running tile index within section
                for qi in range(NQUAD):
                    c0 = (sec * SSEC + qi * QUAD) * 128
                    xT_t = xtpool.tile([D, W4], bf, tag="xT")
                    xdT_t = xtpool.tile([D, W4], bf, tag="xdT")
                    nc.sync.dma_start(out=xT_t[:], in_=xT_d[:, c0:c0 + W4])
                    nc.sync.dma_start(out=xdT_t[:], in_=xdT_d[:, c0:c0 + W4])
                    stage_t = stpool.tile([128, QUAD, D], f32, tag="stage")
                    sums_ps = sumsp.tile([D, 2 * QUAD], f32, tag="sums")

                    agg_ps = aggp.tile([D, W4], f32, tag="agg")
                    for sj in range(QUAD):
                        t_glob = sec * SSEC + qi * QUAD + sj
                        nt = int(ntile[t_glob])
                        for i in range(nt):
                            j = jbase + i
                            oh = ohpool.tile([128, 128], bf, tag="oh")
                            dve = oh_count[0] < OH_DVE_FRAC * oh_total
                            oh_count[0] += 1
                            eng = nc.vector if dve else nc.gpsimd
                            eng.tensor_scalar(oh[:], io_t[:],
                                              dst_t[:, j:j + 1],
                                              nrm_t[:, j:j + 1],
                                              op0=AOp.is_equal, op1=AOp.mult)
                            nc.tensor.matmul(
                                agg_ps[:, sj * 128:(sj + 1) * 128],
                                xe_t[:, j, :], oh[:],
                                start=(i == 0), stop=False)
                        # self-loop injection closes this slot's group:
                        # AggT[:, slot] += I.T @ xdT[:, slot]
                        nc.tensor.matmul(
                            agg_ps[:, sj * 128:(sj + 1) * 128], idb_t[:],
                            xdT_t[:, sj * 128:(sj + 1) * 128],
                            start=False, stop=True)
                        jbase += nt

                    a2_t = wpool.tile([D, W4], bf, tag="a2")
                    nc.scalar.copy(out=a2_t[:], in_=agg_ps[:])
                    g_ps = gp.tile([D, W4], f32, tag="g")
                    nc.tensor.matmul(g_ps[:], Wg_t[:], a2_t[:],
                                     start=True, stop=True)
                    f_ps = fp.tile([D, W4], f32, tag="f")
                    nc.tensor.matmul(f_ps[:], Wf_t[:], xT_t[:],
                                     start=True, stop=True)
                    f_sb = wpool.tile([D, W4], f32, tag="fsb")
                    nc.scalar.activation(f_sb[:], f_ps[:], AF.Prelu,
                                         bias=bf_t[:], scale=1.0, alpha=NEG)
                    sT_sb = wpool.tile([D, W4], f32, tag="st")
                    nc.vector.scalar_tensor_tensor(
                        sT_sb[:], g_ps[:], bg_t[:], f_sb[:],
                        op0=AOp.add, op1=AOp.add)
                    sq_sb = wpool.tile([D, W4], f32, tag="sq")
                    nc.gpsimd.tensor_tensor(sq_sb[:], sT_sb[:], sT_sb[:],
                                            op=AOp.mult)
                    s_ps = sp.tile([128, W4], f32, tag="s")
                    for sj in range(QUAD):
                        sl = slice(sj * 128, (sj + 1) * 128)
                        nc.tensor.transpose(s_ps[:, sl], sT_sb[:, sl], id_t[:])
                        nc.tensor.matmul(sums_ps[:, sj:sj + 1], sT_sb[:, sl],
                                         on_t[:], start=True, stop=True)
                        nc.tensor.matmul(sums_ps[:, QUAD + sj:QUAD + sj + 1],
                                         sq_sb[:, sl], on_t[:],
                                         start=True, stop=True)

                    mean = statp.tile([D, QUAD], f32, tag="mean")
                    nc.vector.tensor_scalar_mul(mean[:], sums_ps[:, :QUAD],
                                                1.0 / D)
                    msq = statp.tile([D, QUAD], f32, tag="msq")
                    nc.vector.tensor_mul(msq[:], mean[:], mean[:])
                    veps = statp.tile([D, QUAD], f32, tag="veps")
                    nc.vector.scalar_tensor_tensor(
                        veps[:], sums_ps[:, QUAD:], 1.0 / D, msq[:],
                        op0=AOp.mult, op1=AOp.subtract)
                    std = statp.tile([D, QUAD], f32, tag="std")
                    nc.scalar.activation(std[:], veps[:], AF.Sqrt, bias=ep_t[:])
                    rstd = statp.tile([D, QUAD], f32, tag="rstd")
                    nc.vector.reciprocal(rstd[:], std[:])
                    negml = statp.tile([D, QUAD], f32, tag="negml")
                    nc.vector.scalar_tensor_tensor(
                        negml[:], mean[:], -1.0, rstd[:],
                        op0=AOp.mult, op1=AOp.mult)

                    for sj in range(QUAD):
                        sl = slice(sj * 128, (sj + 1) * 128)
                        nc.scalar.activation(stage_t[:, sj, :], s_ps[:, sl],
                                             AF.Prelu, bias=negml[:, sj:sj + 1],
                                             scale=rstd[:, sj:sj + 1],
                                             alpha=NEG)

                    r0 = c0
                    n_out = min(W4, max(0, SH - r0))
                    if n_out == W4:
                        nc.sync.dma_start(
                            out=out_d[r0:r0 + W4, :].rearrange(
                                "(j p) d -> p j d", p=128),
                            in_=stage_t[:])
                    else:
                        for sj in range(QUAD):
                            rj = r0 + sj * 128
                            nj = min(128, max(0, SH - rj))
                            if nj > 0:
                                nc.sync.dma_start(out=out_d[rj:rj + nj, :],
                                                  in_=stage_t[:nj, sj, :])
    nc.finalize()
    return nc


def _plan(x_src, x_tar, edge_index, edge_weight, W_gcn, b_gcn, W_fnn, b_fnn):
    """Host prep + (cached) program build. Returns (nc, in_maps, assemble)."""
    bf16 = _bf16()
    x = np.concatenate([np.asarray(x_src, np.float32),
                        np.asarray(x_tar, np.float32)], axis=0)
    ntile, per_core = _host_prep(x, edge_index, edge_weight)

    key = tuple(ntile.tolist())
    if key not in _CACHE:
        _CACHE[key] = _build_program(ntile)
    nc = _CACHE[key]

    iota = np.tile(np.arange(128, dtype=np.float32), (128, 1)).astype(bf16)
    common = {
        "Wg": np.asarray(W_gcn, np.float32).astype(bf16),
        "Wf": np.asarray(W_fnn, np.float32).astype(bf16),
        "bg": np.asarray(b_gcn, np.float32).reshape(D, 1),
        "bf": np.asarray(b_fnn, np.float32).reshape(D, 1),
        "iota": iota,
        "ident": np.eye(D, dtype=np.float32),
        "identb": np.eye(D, dtype=np.float32).astype(bf16),
        "ones": np.ones((D, 1), np.float32),
        "eps": np.full((D, 1), EPS, np.float32),
        "chin": np.zeros((128, 128), np.float32),
    }
    in_maps = []
    for k in range(NCORE):
        m = dict(common)
        m.update(per_core[k])
        in_maps.append(m)

    def assemble(results):
        full = np.concatenate([results[k]["out"] for k in range(NCORE)],
                              axis=0)
        return full[:N_SRC, :], full[N_SRC:, :]

    return nc, in_maps, assemble


def kernel(x_src, x_tar, edge_index, edge_weight, W_gcn, b_gcn, W_fnn, b_fnn):
    from concourse.bass_utils import run_bass_kernel_spmd

    nc, in_maps, assemble = _plan(x_src, x_tar, edge_index, edge_weight,
                                  W_gcn, b_gcn, W_fnn, b_fnn)
    res = run_bass_kernel_spmd(nc, in_maps, list(range(NCORE)))
    return assemble(res.results)


# revision 9
# speedup vs baseline: 2.6683x; 1.0939x over previous
"""GCN message-passing kernel for Trainium2 (8 NeuronCores, Bass/Tile).

Strategy (dest-sharded, host-pregathered edge stream + PE segment-sum):
  - 200k nodes split into 8 shards of 25k destination rows (one per core);
    dest space padded to 25088 = 196 slots of 128 dests.
  - The GCN aggregation A[d] = sum_e norm_e x[src_e] is factored as
    norm_e = dinv[src] * w_e * dinv[dst]: dinv[src] is folded into a host
    tensor xs = dinv*x (bf16); w_e*dinv[dst] becomes the value of a per-tile
    one-hot matrix; the host pre-gathers xe[i] = xs[src_e] in (core, slot)-
    sorted order so the device streams it contiguously (no dma_gather).
    Each slot's messages are padded to whole 128-row tiles; tile counts are
    shared across cores (max over cores) so the SPMD program is uniform.
  - Per 128-msg tile, DVE/Pool builds onehot[m, d] = (iota==dst_m)*nrm_m
    (bf16); PE accumulates AggT[feat, dst] += xe_tile.T @ onehot into a
    per-quad (4 slots) PSUM bank.
  - Phase C per quad (feature-major): A2T = AggT + xdT (self loops,
    xdT = (dinv^2 x)^T), gT = Wg.T@A2T, fT = Wf.T@xT, f = Prelu(fT+bf),
    sT = gT+bg+f, s = sT.T per slot (PE transpose), row sums of s and s^2
    via ones-matmuls (s^2 on Pool), out = Prelu(rstd*s - rstd*mean).
"""

import numpy as np

N_SRC = 100000
N_TAR = 100000
N = N_SRC + N_TAR
D = 128
NCORE = 8
SH = N // NCORE          # 25000 dest rows per core
NSLOT = 196              # slots of 128 dests; 196*128 = 25088 >= 25000
SPAD = NSLOT * 128
NSEC = 7                 # sections of 28 slots (xe streaming granularity)
SSEC = NSLOT // NSEC     # 28 slots per section
QUAD = 4                 # slots per PSUM quad
NQUAD = SSEC // QUAD     # 7 quads per section
EPS = 1e-6
NEG = 0.01
OH_DVE_FRAC = 0.75       # fraction of one-hot builds on DVE (rest on Pool)

_CACHE = {}


def _bf16():
    import ml_dtypes
    return ml_dtypes.bfloat16


def _host_prep(x, edge_index, edge_weight):
    """Returns (ntile [NSLOT], per_core in_map fragments)."""
    bf16 = _bf16()
    row = np.asarray(edge_index[0], dtype=np.int64)
    col = np.asarray(edge_index[1], dtype=np.int64)
    w = np.asarray(edge_weight, dtype=np.float32)

    deg = np.bincount(col, weights=w.astype(np.float64), minlength=N)
    deg = (deg + 1.0).astype(np.float32)
    dinv = (1.0 / np.sqrt(deg)).astype(np.float32)

    xs16 = (x * dinv[:, None]).astype(bf16)          # dinv[src] folded in
    nrm2 = (w * dinv[col]).astype(np.float32)        # per-edge w_e*dinv[dst]

    core = col // SH
    dstl = col - core * SH
    slot = dstl >> 7                                 # 0..195
    dis = (dstl & 127).astype(np.float32)            # dest-in-slot

    key = core * NSLOT + slot
    cnt = np.bincount(key, minlength=NCORE * NSLOT).reshape(NCORE, NSLOT)
    cap = np.maximum(cnt.max(axis=0), 1).astype(np.int64)
    ntile = (cap + 127) // 128                       # tiles per slot (shared)
    capp = ntile * 128

    # slot start offsets (tile-aligned) within each section
    o_slot = np.zeros(NSLOT, dtype=np.int64)
    B_list = []
    for sec in range(NSEC):
        cq = capp[sec * SSEC:(sec + 1) * SSEC]
        off = np.concatenate([[0], np.cumsum(cq)])
        o_slot[sec * SSEC:(sec + 1) * SSEC] = off[:-1]
        B_list.append(int(off[-1]))

    # rank of each edge within its (core, slot) bucket
    order = np.argsort(key, kind="stable")
    ks = key[order]
    change = np.empty(len(ks), dtype=bool)
    change[0] = True
    change[1:] = ks[1:] != ks[:-1]
    runstart = np.maximum.accumulate(np.where(change, np.arange(len(ks)), 0))
    rank = np.arange(len(ks)) - runstart
    rank_e = np.empty(len(ks), dtype=np.int64)
    rank_e[order] = rank

    pos = o_slot[slot] + rank_e                      # row within section buf
    secs = slot // SSEC

    per_core = []
    for k in range(NCORE):
        mk = core == k
        m = {}
        for sec in range(NSEC):
            sel = mk & (secs == sec)
            B = B_list[sec]
            T = B // 128
            xe = np.zeros((B, D), dtype=bf16)
            xe[pos[sel]] = xs16[row[sel]]
            dstb = np.zeros(B, dtype=np.float32)
            dstb[pos[sel]] = dis[sel]
            nrmb = np.zeros(B, dtype=np.float32)
            nrmb[pos[sel]] = nrm2[sel]
            m[f"xe{sec}"] = np.ascontiguousarray(
                xe.reshape(T, 128, D).transpose(1, 0, 2).reshape(128, T * D))
            m[f"dst{sec}"] = np.ascontiguousarray(dstb.reshape(T, 128).T)
            m[f"nrm{sec}"] = np.ascontiguousarray(nrmb.reshape(T, 128).T)
        xk = x[k * SH:(k + 1) * SH]
        d2k = (dinv[k * SH:(k + 1) * SH] ** 2).astype(np.float32)
        xT = np.zeros((D, SPAD), dtype=bf16)
        xT[:, :SH] = xk.T.astype(bf16)
        xdT = np.zeros((D, SPAD), dtype=bf16)
        xdT[:, :SH] = (xk * d2k[:, None]).T.astype(bf16)
        m["xT"] = xT
        m["xdT"] = xdT
        per_core.append(m)

    return ntile, per_core


def _build_program(ntile):
    from concourse import bacc, mybir, tile

    f32 = mybir.dt.float32
    bf = mybir.dt.bfloat16
    nc = bacc.Bacc(None)

    T_list = [int(ntile[s * SSEC:(s + 1) * SSEC].sum()) for s in range(NSEC)]
    xe_d = [nc.dram_tensor(f"xe{s}", [128, T_list[s] * D], bf,
                           kind="ExternalInput") for s in range(NSEC)]
    dst_d = [nc.dram_tensor(f"dst{s}", [128, T_list[s]], f32,
                            kind="ExternalInput") for s in range(NSEC)]
    nrm_d = [nc.dram_tensor(f"nrm{s}", [128, T_list[s]], f32,
                            kind="ExternalInput") for s in range(NSEC)]
    xT_d = nc.dram_tensor("xT", [D, SPAD], bf, kind="ExternalInput")
    xdT_d = nc.dram_tensor("xdT", [D, SPAD], bf, kind="ExternalInput")
    Wg_d = nc.dram_tensor("Wg", [D, D], bf, kind="ExternalInput")
    Wf_d = nc.dram_tensor("Wf", [D, D], bf, kind="ExternalInput")
    bg_d = nc.dram_tensor("bg", [D, 1], f32, kind="ExternalInput")
    bf_d = nc.dram_tensor("bf", [D, 1], f32, kind="ExternalInput")
    io_d = nc.dram_tensor("iota", [128, 128], bf, kind="ExternalInput")
    id_d = nc.dram_tensor("ident", [D, D], f32, kind="ExternalInput")
    idb_d = nc.dram_tensor("identb", [D, D], bf, kind="ExternalInput")
    on_d = nc.dram_tensor("ones", [D, 1], f32, kind="ExternalInput")
    ep_d = nc.dram_tensor("eps", [D, 1], f32, kind="ExternalInput")
    out_d = nc.dram_tensor("out", [SH, D], f32, kind="ExternalOutput")
    chin_d = nc.dram_tensor("chin", [128, 128], f32, kind="ExternalInput")
    chout_d = nc.dram_tensor("chout", [128, 128], f32, kind="ExternalOutput")

    AOp = mybir.AluOpType
    AF = mybir.ActivationFunctionType
    W4 = QUAD * 128      # 512

    oh_count = [0]
    oh_total = int(ntile.sum())

    with tile.TileContext(nc) as tc:
        with tc.tile_pool(name="const", bufs=1) as cpool, \
             tc.tile_pool(name="xep", bufs=2) as xepool, \
             tc.tile_pool(name="metap", bufs=2) as mpool, \
             tc.tile_pool(name="ohp", bufs=4) as ohpool, \
             tc.tile_pool(name="xtp", bufs=2) as xtpool, \
             tc.tile_pool(name="work", bufs=2) as wpool, \
             tc.tile_pool(name="stagep", bufs=2) as stpool, \
             tc.tile_pool(name="statp", bufs=2) as statp, \
             tc.tile_pool(name="aggp", bufs=2, space="PSUM") as aggp, \
             tc.tile_pool(name="gp", bufs=2, space="PSUM") as gp, \
             tc.tile_pool(name="fp", bufs=1, space="PSUM") as fp, \
             tc.tile_pool(name="sp", bufs=2, space="PSUM") as sp, \
             tc.tile_pool(name="sumsp", bufs=1, space="PSUM") as sumsp:

            Wg_t = cpool.tile([D, D], bf, tag="wg")
            Wf_t = cpool.tile([D, D], bf, tag="wf")
            bg_t = cpool.tile([D, 1], f32, tag="bg")
            bf_t = cpool.tile([D, 1], f32, tag="bf")
            io_t = cpool.tile([128, 128], bf, tag="io")
            id_t = cpool.tile([D, D], f32, tag="id")
            idb_t = cpool.tile([D, D], bf, tag="idb")
            on_t = cpool.tile([D, 1], f32, tag="on")
            ep_t = cpool.tile([D, 1], f32, tag="ep")
            for t, d in [(Wg_t, Wg_d), (Wf_t, Wf_d), (bg_t, bg_d),
                         (bf_t, bf_d), (io_t, io_d), (id_t, id_d),
                         (idb_t, idb_d), (on_t, on_d), (ep_t, ep_d)]:
                nc.sync.dma_start(out=t[:], in_=d[:])
            ch_t = cpool.tile([128, 128], f32, tag="ch")
            nc.sync.dma_start(out=ch_t[:], in_=chin_d[:])
            nc.sync.dma_start(out=chout_d[:], in_=ch_t[:])

            for sec in range(NSEC):
                T = T_list[sec]
                xe_t = xepool.tile([128, T, D], bf, tag="xe")
                nc.sync.dma_start(out=xe_t[:], in_=xe_d[sec][:].rearrange(
                    "p (t d) -> p t d", d=D))
                dst_t = mpool.tile([128, T], f32, tag="dst")
                nrm_t = mpool.tile([128, T], f32, tag="nrm")
                nc.sync.dma_start(out=dst_t[:], in_=dst_d[sec][:])
                nc.sync.dma_start(out=nrm_t[:], in_=nrm_d[sec][:])

                s0 = sec * SSEC * 128
                WS = SSEC * 128
                xT_t = xtpool.tile([D, WS], bf, tag="xT")
                xdT_t = xtpool.tile([D, WS], bf, tag="xdT")
                nc.sync.dma_start(out=xT_t[:], in_=xT_d[:, s0:s0 + WS])
                nc.sync.dma_start(out=xdT_t[:], in_=xdT_d[:, s0:s0 + WS])

                jbase = 0           # running tile index within section
                stage_t = None
                for qi in range(NQUAD):
                    c0 = (sec * SSEC + qi * QUAD) * 128
                    q0 = qi * QUAD * 128      # quad base within section
                    if qi % 2 == 0:
                        stage_t = stpool.tile([128, 2 * QUAD, D], f32,
                                              tag="stage")
                    sg = (qi % 2) * QUAD      # quad offset within stage
                    sums_ps = sumsp.tile([D, 2 * QUAD], f32, tag="sums")

                    agg_ps = aggp.tile([D, W4], f32, tag="agg")
                    for sj in range(QUAD):
                        t_glob = sec * SSEC + qi * QUAD + sj
                        nt = int(ntile[t_glob])
                        for i in range(nt):
                            j = jbase + i
                            oh = ohpool.tile([128, 128], bf, tag="oh")
                            dve = oh_count[0] < OH_DVE_FRAC * oh_total
                            oh_count[0] += 1
                            eng = nc.vector if dve else nc.gpsimd
                            eng.tensor_scalar(oh[:], io_t[:],
                                              dst_t[:, j:j + 1],
                                              nrm_t[:, j:j + 1],
                                              op0=AOp.is_equal, op1=AOp.mult)
                            nc.tensor.matmul(
                                agg_ps[:, sj * 128:(sj + 1) * 128],
                                xe_t[:, j, :], oh[:],
                                start=(i == 0), stop=False)
                        # self-loop injection closes this slot's group:
                        # AggT[:, slot] += I.T @ xdT[:, slot]
                        nc.tensor.matmul(
                            agg_ps[:, sj * 128:(sj + 1) * 128], idb_t[:],
                            xdT_t[:, q0 + sj * 128:q0 + (sj + 1) * 128],
                            start=False, stop=True)
                        jbase += nt

                    a2_t = wpool.tile([D, W4], bf, tag="a2")
                    nc.scalar.copy(out=a2_t[:], in_=agg_ps[:])
                    g_ps = gp.tile([D, W4], f32, tag="g")
                    nc.tensor.matmul(g_ps[:], Wg_t[:], a2_t[:],
                                     start=True, stop=True)
                    f_ps = fp.tile([D, W4], f32, tag="f")
                    nc.tensor.matmul(f_ps[:], Wf_t[:], xT_t[:, q0:q0 + W4],
                                     start=True, stop=True)
                    f_sb = wpool.tile([D, W4], f32, tag="fsb")
                    nc.scalar.activation(f_sb[:], f_ps[:], AF.Prelu,
                                         bias=bf_t[:], scale=1.0, alpha=NEG)
                    sT_sb = wpool.tile([D, W4], f32, tag="st")
                    nc.vector.scalar_tensor_tensor(
                        sT_sb[:], g_ps[:], bg_t[:], f_sb[:],
                        op0=AOp.add, op1=AOp.add)
                    sq_sb = wpool.tile([D, W4], f32, tag="sq")
                    nc.gpsimd.tensor_tensor(sq_sb[:], sT_sb[:], sT_sb[:],
                                            op=AOp.mult)
                    s_ps = sp.tile([128, W4], f32, tag="s")
                    for sj in range(QUAD):
                        sl = slice(sj * 128, (sj + 1) * 128)
                        nc.tensor.transpose(s_ps[:, sl], sT_sb[:, sl], id_t[:])
                        nc.tensor.matmul(sums_ps[:, sj:sj + 1], sT_sb[:, sl],
                                         on_t[:], start=True, stop=True)
                        nc.tensor.matmul(sums_ps[:, QUAD + sj:QUAD + sj + 1],
                                         sq_sb[:, sl], on_t[:],
                                         start=True, stop=True)

                    mean = statp.tile([D, QUAD], f32, tag="mean")
                    nc.vector.tensor_scalar_mul(mean[:], sums_ps[:, :QUAD],
                                                1.0 / D)
                    msq = statp.tile([D, QUAD], f32, tag="msq")
                    nc.vector.tensor_mul(msq[:], mean[:], mean[:])
                    veps = statp.tile([D, QUAD], f32, tag="veps")
                    nc.vector.scalar_tensor_tensor(
                        veps[:], sums_ps[:, QUAD:], 1.0 / D, msq[:],
                        op0=AOp.mult, op1=AOp.subtract)
                    std = statp.tile([D, QUAD], f32, tag="std")
                    nc.scalar.activation(std[:], veps[:], AF.Sqrt, bias=ep_t[:])
                    rstd = statp.tile([D, QUAD], f32, tag="rstd")
                    nc.vector.reciprocal(rstd[:], std[:])
                    negml = statp.tile([D, QUAD], f32, tag="negml")
                    nc.vector.scalar_tensor_tensor(
                        negml[:], mean[:], -1.0, rstd[:],
                        op0=AOp.mult, op1=AOp.mult)

                    for sj in range(QUAD):
                        sl = slice(sj * 128, (sj + 1) * 128)
                        nc.scalar.activation(stage_t[:, sg + sj, :],
                                             s_ps[:, sl],
                                             AF.Prelu, bias=negml[:, sj:sj + 1],
                                             scale=rstd[:, sj:sj + 1],
                                             alpha=NEG)

                    if qi % 2 == 1 or qi == NQUAD - 1:
                        nq = sg + QUAD            # quads in this stage
                        r0 = c0 - (sg // QUAD) * W4
                        n_out = min(nq * 128, max(0, SH - r0))
                        if n_out == nq * 128:
                            nc.sync.dma_start(
                                out=out_d[r0:r0 + nq * 128, :].rearrange(
                                    "(j p) d -> p j d", p=128),
                                in_=stage_t[:, :nq, :])
                        else:
                            for sj in range(nq):
                                rj = r0 + sj * 128
                                nj = min(128, max(0, SH - rj))
                                if nj > 0:
                                    nc.sync.dma_start(
                                        out=out_d[rj:rj + nj, :],
                                        in_=stage_t[:nj, sj, :])
    nc.finalize()
    return nc


def _plan(x_src, x_tar, edge_index, edge_weight, W_gcn, b_gcn, W_fnn, b_fnn):
    """Host prep + (cached) program build. Returns (nc, in_maps, assemble)."""
    bf16 = _bf16()
    x = np.concatenate([np.asarray(x_src, np.float32),
                        np.asarray(x_tar, np.float32)], axis=0)
    ntile, per_core = _host_prep(x, edge_index, edge_weight)

    key = tuple(ntile.tolist())
    if key not in _CACHE:
        _CACHE[key] = _build_program(ntile)
    nc = _CACHE[key]

    iota = np.tile(np.arange(128, dtype=np.float32), (128, 1)).astype(bf16)
    common = {
        "Wg": np.asarray(W_gcn, np.float32).astype(bf16),
        "Wf": np.asarray(W_fnn, np.float32).astype(bf16),
        "bg": np.asarray(b_gcn, np.float32).reshape(D, 1),
        "bf": np.asarray(b_fnn, np.float32).reshape(D, 1),
        "iota": iota,
        "ident": np.eye(D, dtype=np.float32),
        "identb": np.eye(D, dtype=np.float32).astype(bf16),
        "ones": np.ones((D, 1), np.float32),
        "eps": np.full((D, 1), EPS, np.float32),
        "chin": np.zeros((128, 128), np.float32),
    }
    in_maps = []
    for k in range(NCORE):
        m = dict(common)
        m.update(per_core[k])
        in_maps.append(m)

    def assemble(results):
        full = np.concatenate([results[k]["out"] for k in range(NCORE)],
                              axis=0)
        return full[:N_SRC, :], full[N_SRC:, :]

    return nc, in_maps, assemble


def kernel(x_src, x_tar, edge_index, edge_weight, W_gcn, b_gcn, W_fnn, b_fnn):
    from concourse.bass_utils import run_bass_kernel_spmd

    nc, in_maps, assemble = _plan(x_src, x_tar, edge_index, edge_weight,
                                  W_gcn, b_gcn, W_fnn, b_fnn)
    res = run_bass_kernel_spmd(nc, in_maps, list(range(NCORE)))
    return assemble(res.results)
